# revision 1
# baseline (speedup 1.0000x reference)
"""Trainium2 Bass kernel for nn_CrowdsClassificationSModel.

Reference computation:
    W = softmax(kernel, axis=1)            # (8, 8, 59)
    out = einsum('bc,cdr->bdr', x, W)      # (131072, 8, 59)
    out = where(drop_mask, out / 0.6, 0)

Strategy (data-parallel over 8 NeuronCores, batch-sharded; per core
bc=16384 batches, b = p*128 + n for partition p, n in [0,128)):

  - Host computes W = softmax(kernel)/keep and splits it into a bf16
    hi/lo pair (Wh + Wl ~= W).  x is split on-device the same way
    (xh + xl ~= x), giving a 3-term bf16 product
        out ~= xh@Wh + xl@Wh + xh@Wl        (error ~ 2^-18)
    evaluated as ONE K=96 bf16 matmul per 128-batch tile: lhsT rows =
    [xhT; xlT; xhT] blocks, rhs rows = [Wh; Wh; Wl] blocks.  Each
    32-row block is zero-padded so one of 4 interleaved batch
    sub-tiles (n_local = k) is selected by the rhs variant.
  - lhsT blocks come from PE transposes of 32-column slices of the
    [128, 1024] x image into one [96, 128] PSUM tile, copied to SBUF
    by the Scalar engine.
  - dropout: DVE multiplies PSUM matmul results by the uint8 keep mask
    (0/1) into SBUF staging; matmuls write two 472-wide results into
    one 2-bank PSUM tile so each DVE op covers 944 elements.
  - supertile = 4 n-values: mask in / out DMA lines are 4*472 elems
    per partition (1888 B / 7552 B contiguous).
"""

import numpy as np

import concourse.bacc as bacc
import concourse.bass as bass
import concourse.tile as tile
from concourse import mybir
from concourse.bass_utils import run_bass_kernel_spmd

N_CORES = 8
B_FULL = 131072
C = 8
R = 59
F = C * R  # 472
DROP_RATE = 0.4
KEEP = np.float32(1.0 - DROP_RATE)
NT = 4  # batch sub-tiles per supertile
import os as _os

MODE = _os.environ.get("KMODE", "bf16x3")  # or "f32r"


def softmax_np(k: np.ndarray, axis: int) -> np.ndarray:
    k = k.astype(np.float64)
    m = k.max(axis=axis, keepdims=True)
    e = np.exp(k - m)
    return (e / e.sum(axis=axis, keepdims=True)).astype(np.float32)


def _bf16_split(a: np.ndarray):
    import ml_dtypes

    hi = a.astype(ml_dtypes.bfloat16)
    lo = (a - hi.astype(np.float32)).astype(ml_dtypes.bfloat16)
    return hi, lo


def build_w(kernel: np.ndarray) -> np.ndarray:
    """(8,8,59) raw kernel -> zero-padded, keep-scaled rhs blocks.

    bf16x3: (96, NT*472) bf16, row blocks [Wh; Wh; Wl], sub-tile k at
    rows 8k..8k+8 of each block.
    f32r:   (32, NT*472) f32.
    """
    w = softmax_np(kernel, axis=1).reshape(C, F) / KEEP  # (8, 472) f32
    if MODE == "f32r":
        w32 = np.zeros((4 * C, NT * F), dtype=np.float32)
        for k in range(NT):
            w32[8 * k : 8 * (k + 1), k * F : (k + 1) * F] = w
        return w32
    import ml_dtypes

    wh, wl = _bf16_split(w)
    out = np.zeros((3 * 4 * C, NT * F), dtype=ml_dtypes.bfloat16)
    for b, blk in enumerate((wh, wh, wl)):
        for k in range(NT):
            out[32 * b + 8 * k : 32 * b + 8 * (k + 1), k * F : (k + 1) * F] = blk
    return out


def build_module(bc: int) -> bass.Bass:
    assert bc % (128 * NT) == 0
    n_total = bc // 128
    n_super = n_total // NT
    fs = NT * F

    nc = bacc.Bacc("TRN2", target_bir_lowering=False, debug=False)
    f32 = mybir.dt.float32
    f32r = mybir.dt.float32r
    bf16 = mybir.dt.bfloat16
    u8 = mybir.dt.uint8

    if MODE == "f32r":
        mm_dt, kk, x_dt = f32r, 4 * C, f32r
    else:
        mm_dt, kk, x_dt = bf16, 3 * 4 * C, f32

    x_d = nc.dram_tensor("x_sh", (bc, C), x_dt, kind="ExternalInput")
    m_d = nc.dram_tensor("mask_sh", (bc, F), u8, kind="ExternalInput")
    w_d = nc.dram_tensor("w_blk", (kk, fs), mm_dt, kind="ExternalInput")
    o_d = nc.dram_tensor("out_sh", (bc, F), f32, kind="ExternalOutput")

    m_view = m_d[:].rearrange("(p s k) f -> s p (k f)", p=128, s=n_super, k=NT)
    o_view = o_d[:].rearrange("(p s k) f -> s p (k f)", p=128, s=n_super, k=NT)
    x_view = x_d[:].rearrange("(p n) c -> p (n c)", p=128)

    id_d = nc.dram_tensor("ident128", (128, 128), mm_dt, kind="ExternalInput")

    xw = n_total * C  # x image free width (1024)
    n_xchunk = min(4, n_super)
    cw = xw // n_xchunk
    spc = n_super // n_xchunk  # supertiles per x chunk

    with tile.TileContext(nc) as tc:
        with (
            tc.tile_pool(name="const", bufs=1) as constp,
            tc.tile_pool(name="xl", bufs=1) as xlp,
            tc.tile_pool(name="xt", bufs=4) as xtp,
            tc.tile_pool(name="mask", bufs=8) as maskp,
            tc.tile_pool(name="stage", bufs=8) as stagep,
            tc.tile_pool(name="pt", bufs=2, space="PSUM") as ptp,
            tc.tile_pool(name="pmm", bufs=6, space="PSUM") as pmmp,
        ):
            ident = constp.tile([128, 128], mm_dt)
            nc.sync.dma_start(ident[:], id_d[:])

            x_l = xlp.tile([128, xw], x_dt)
            if MODE == "bf16x3":
                # interleaved bf16 image: per supertile s, 96 columns
                # [xh(32) | xl(32) | xh(32)] -> one base-0 transpose
                xi_l = xlp.tile([128, 3 * xw], bf16)
                xi_v = xi_l[:].rearrange("p (s b) -> p s b", b=96)
                xh_f = xlp.tile([128, xw], f32)
            for q in range(n_xchunk):
                sl = slice(q * cw, (q + 1) * cw)
                nc.sync.dma_start(x_l[:, sl], x_view[:, sl])
                if MODE == "bf16x3":
                    ssl = slice(q * spc, (q + 1) * spc)
                    x_v = x_l[:, sl].rearrange("p (s b) -> p s b", b=32)
                    nc.vector.tensor_copy(xi_v[:, ssl, 0:32], x_v)
                    nc.vector.tensor_copy(xi_v[:, ssl, 64:96], x_v)
                    xh_v = xh_f[:, sl].rearrange("p (s b) -> p s b", b=32)
                    nc.vector.tensor_copy(xh_v, xi_v[:, ssl, 0:32])
                    nc.vector.tensor_sub(xi_v[:, ssl, 32:64], x_v, xh_v)
                if q == 0:
                    w_t = constp.tile([kk, fs], mm_dt)
                    nc.sync.dma_start(w_t[:], w_d[:])

            for s in range(n_super):
                pt = ptp.tile([kk, 128], mm_dt)
                if MODE == "f32r":
                    nc.tensor.transpose(
                        pt[:], x_l[:, s * 32 : (s + 1) * 32], ident[:]
                    )
                else:
                    nc.tensor.transpose(
                        pt[:], xi_l[:, s * 96 : (s + 1) * 96], ident[:]
                    )
                xt = xtp.tile([kk, 128], mm_dt)
                nc.scalar.copy(xt[:], pt[:])

                mt = maskp.tile([128, fs], u8)
                nc.scalar.dma_start(mt[:], m_view[s])

                st = stagep.tile([128, fs], f32)
                for k in range(NT):
                    pm = pmmp.tile([128, F], f32)
                    nc.tensor.matmul(
                        pm[:],
                        xt[:],
                        w_t[:, k * F : (k + 1) * F],
                        start=True,
                        stop=True,
                    )
                    nc.vector.tensor_mul(
                        st[:, k * F : (k + 1) * F],
                        pm[:],
                        mt[:, k * F : (k + 1) * F],
                    )

                nc.sync.dma_start(o_view[s], st[:])

    nc.compile()
    return nc


_CACHE: dict = {}


def _get_module(bc: int):
    if bc not in _CACHE:
        _CACHE[bc] = build_module(bc)
    return _CACHE[bc]


def _prep_inputs(x, kernel, drop_mask, bc):
    w_blk = build_w(np.asarray(kernel))
    x = np.ascontiguousarray(np.asarray(x, dtype=np.float32))
    mask = np.asarray(drop_mask)
    if mask.dtype != np.uint8:
        mask = mask.astype(np.uint8)
    mask = np.ascontiguousarray(mask.reshape(mask.shape[0], -1))
    ident = np.eye(128, dtype=w_blk.dtype)
    n_shards = x.shape[0] // bc
    in_maps = []
    for i in range(n_shards):
        in_maps.append(
            {
                "x_sh": x[i * bc : (i + 1) * bc],
                "mask_sh": mask[i * bc : (i + 1) * bc],
                "w_blk": w_blk,
                "ident128": ident,
            }
        )
    return in_maps


def run(x, kernel, drop_mask, trace: bool = False):
    bc = x.shape[0] // N_CORES
    nc = _get_module(bc)
    in_maps = _prep_inputs(x, kernel, drop_mask, bc)
    res = run_bass_kernel_spmd(
        nc, in_maps, core_ids=list(range(N_CORES)), trace=trace
    )
    out = np.concatenate([r["out_sh"] for r in res.results], axis=0)
    return out.reshape(B_FULL, C, R), res


def kernel(x, kernel, drop_mask) -> np.ndarray:
    out, _ = run(x, kernel, drop_mask, trace=False)
    return out

